# revision 3
# baseline (speedup 1.0000x reference)
"""Trainium2 Bass kernel for the LoRA-with-conditional-gating dense MLP.

Math (per batch element b):
    h        = LayerNorm(ctr_hidden[b]) * ln_gamma + ln_beta
    f        = h @ W_ctr.T + b_ctr                        # [CTR_F]
    sA       = f @ W_A_adapter.T                          # [R]
    sB       = f @ W_B_adapter.T                          # [D_OUT]
    a        = x[b] @ W_A.T                               # [S, R]
    out[b]   = (a * sA) @ W_B.T * sB * SCALING            # [S, D_OUT]

Both gates and the scaling fold into a tiny per-batch effective weight:
    W_eff.T[r, o] = SCALING * sA[r] * W_B[o, r] * sB[o]   # [R, D_OUT]
    out[b] = (x[b] @ W_A.T) @ W_eff.T

The scalar path (LayerNorm + three tiny matvecs, ~1.4 MFLOP total) is
computed on the host in float64; the device kernel does the two big
matmuls (21.5 GFLOP) and moves the 640 MiB of x/out traffic.

Sharding: pure data-parallel over B=8 across the 8 NeuronCores (one
batch element per core, no collectives). Per core:
  - PE transposes x tiles (128x128, via identity matmul) so the d=5120
    contraction lands on the partition axis,
  - mm1: aT[r, bs] += W_A.T[d-chunk].T-contraction over 40 d-chunks,
  - mm2: out[bs, o] = aT.T @ W_eff.T in 512-wide o chunks,
  - DVE/ACT copy PSUM->SBUF, HWDGE/SWDGE DMA the 2.5 MiB row tiles.
All fp32 end to end.
"""

from contextlib import ExitStack

import numpy as np

# Problem shape (hardcoded per harness contract).
B, S = 8, 2048
D_IN = 5120
D_OUT = 5120
R = 64
CTR_H = 256
CTR_F = 128
ALPHA = 128.0
SCALING = ALPHA / R
LN_EPS = 1e-5

N_CORES = 8
P = 128                    # partitions
DCH = D_IN // P            # 40 d-chunks of 128
BS_BLK = 256               # bs rows per mm1 block (moving free dim)
N_BLK = S // BS_BLK        # 8
N_TILE = S // P            # 16 row tiles of 128
O_CH = 512                 # output chunk (one PSUM bank of fp32)
N_OCH = D_OUT // O_CH      # 10

_NC_CACHE = {}


def _build_nc(chain=1):
    """Build + compile the single-core SPMD Bass program (cached).

    chain > 1 wraps the whole body in a hardware For_i loop that re-runs
    it `chain` times — used by the timing harness to isolate device-exec
    time from host/RPC overhead. The graded path uses chain=1.
    """
    if chain in _NC_CACHE:
        return _NC_CACHE[chain]

    import concourse.bacc as bacc
    import concourse.mybir as mybir
    import concourse.tile as tile
    from concourse import masks

    nc = bacc.Bacc("TRN2", target_bir_lowering=False, debug=False,
                   num_devices=N_CORES)
    f32 = mybir.dt.float32

    x_d = nc.dram_tensor("x", [S, D_IN], f32, kind="ExternalInput")
    wa_d = nc.dram_tensor("wa_t", [P, DCH * R], f32, kind="ExternalInput")
    weff_d = nc.dram_tensor("weff_t", [R, D_OUT], f32, kind="ExternalInput")
    out_d = nc.dram_tensor("out", [S, D_OUT], f32, kind="ExternalOutput")

    with tile.TileContext(nc) as tc, ExitStack() as ctx:
        const = ctx.enter_context(tc.tile_pool(name="const", bufs=1))
        x_pool = ctx.enter_context(tc.tile_pool(name="x_nat", bufs=3))
        xt_pool = ctx.enter_context(tc.tile_pool(name="xt", bufs=3))
        at_pool = ctx.enter_context(tc.tile_pool(name="at", bufs=2))
        out_pool = ctx.enter_context(tc.tile_pool(name="out_sb", bufs=2))
        ps_xt = ctx.enter_context(tc.tile_pool(name="ps_xt", bufs=3, space="PSUM"))
        ps_a = ctx.enter_context(tc.tile_pool(name="ps_a", bufs=2, space="PSUM"))
        ps_o = ctx.enter_context(tc.tile_pool(name="ps_o", bufs=3, space="PSUM"))

        ident = const.tile([P, P], f32)
        masks.make_identity(nc, ident[:])
        wa_sb = const.tile([P, DCH * R], f32)
        nc.sync.dma_start(out=wa_sb[:], in_=wa_d[:])
        weff_sb = const.tile([R, D_OUT], f32)
        nc.sync.dma_start(out=weff_sb[:], in_=weff_d[:])

        loop_ctx = tc.For_i(0, chain, 1) if chain > 1 else None
        if loop_ctx is not None:
            ctx.enter_context(loop_ctx)

        for blk in range(N_BLK):
            xn0 = x_pool.tile([P, D_IN], f32, tag="x_nat")
            nc.sync.dma_start(out=xn0[:], in_=x_d[blk * BS_BLK: blk * BS_BLK + P, :])
            xn1 = x_pool.tile([P, D_IN], f32, tag="x_nat")
            nc.sync.dma_start(out=xn1[:], in_=x_d[blk * BS_BLK + P: blk * BS_BLK + 2 * P, :])

            pa = ps_a.tile([R, BS_BLK], f32)
            for d in range(DCH):
                pxt = ps_xt.tile([P, BS_BLK], f32)
                nc.tensor.transpose(pxt[:, 0:P], xn0[:, d * P:(d + 1) * P], ident[:])
                nc.tensor.transpose(pxt[:, P:2 * P], xn1[:, d * P:(d + 1) * P], ident[:])
                xt = xt_pool.tile([P, BS_BLK], f32, tag="xt")
                cp = nc.vector.tensor_copy if d % 2 == 0 else nc.scalar.copy
                cp(xt[:], pxt[:])
                nc.tensor.matmul(pa[:], wa_sb[:, d * R:(d + 1) * R], xt[:],
                                 start=(d == 0), stop=(d == DCH - 1))

            at = at_pool.tile([R, BS_BLK], f32, tag="at")
            nc.vector.tensor_copy(at[:], pa[:])

            for t in range(2):
                row0 = blk * BS_BLK + t * P
                osb = out_pool.tile([P, D_OUT], f32, tag="out_sb")
                for o in range(N_OCH):
                    po = ps_o.tile([P, O_CH], f32)
                    nc.tensor.matmul(po[:], at[:, t * P:(t + 1) * P],
                                     weff_sb[:, o * O_CH:(o + 1) * O_CH],
                                     start=True, stop=True)
                    cp = nc.scalar.copy if o % 2 == 0 else nc.vector.tensor_copy
                    cp(osb[:, o * O_CH:(o + 1) * O_CH], po[:])
                nc.gpsimd.dma_start(out=out_d[row0: row0 + P, :], in_=osb[:])

    nc.compile()
    _NC_CACHE[chain] = nc
    return nc


def _host_prep(ctr_hidden, ln_gamma, ln_beta, W_ctr, b_ctr,
               W_A_adapter, W_B_adapter, W_A, W_B):
    """Scalar path in float64; returns packed W_A.T and per-batch W_eff.T."""
    ch = np.asarray(ctr_hidden, dtype=np.float64)
    mu = ch.mean(axis=-1, keepdims=True)
    var = ((ch - mu) ** 2).mean(axis=-1, keepdims=True)
    h = (ch - mu) / np.sqrt(var + LN_EPS)
    h = h * np.asarray(ln_gamma, np.float64) + np.asarray(ln_beta, np.float64)
    f = h @ np.asarray(W_ctr, np.float64).T + np.asarray(b_ctr, np.float64)
    sA = f @ np.asarray(W_A_adapter, np.float64).T            # [B, R]
    sB = f @ np.asarray(W_B_adapter, np.float64).T            # [B, D_OUT]

    wbt = np.asarray(W_B, np.float64).T                       # [R, D_OUT]
    weff_t = (SCALING * sA[:, :, None] * wbt[None] * sB[:, None, :])
    weff_t = np.ascontiguousarray(weff_t, dtype=np.float32)   # [B, R, D_OUT]

    wa_t = np.asarray(W_A, np.float32).T                      # [D_IN, R]
    wa_packed = np.ascontiguousarray(
        wa_t.reshape(DCH, P, R).transpose(1, 0, 2).reshape(P, DCH * R))
    return wa_packed, weff_t


def kernel(x, ctr_hidden, ln_gamma, ln_beta, W_ctr, b_ctr,
           W_A_adapter, W_B_adapter, W_A, W_B):
    from concourse import bass_utils

    x = np.asarray(x, dtype=np.float32)
    wa_packed, weff_t = _host_prep(ctr_hidden, ln_gamma, ln_beta, W_ctr, b_ctr,
                                   W_A_adapter, W_B_adapter, W_A, W_B)

    nc = _build_nc()
    in_maps = [
        {
            "x": np.ascontiguousarray(x[b]),
            "wa_t": wa_packed,
            "weff_t": weff_t[b],
        }
        for b in range(B)
    ]
    res = bass_utils.run_bass_kernel_spmd(nc, in_maps, list(range(N_CORES)))
    return np.stack([res.results[b]["out"] for b in range(B)]).astype(np.float32)
